# revision 6
# baseline (speedup 1.0000x reference)
"""Trainium2 Bass kernel for the CO2-electrolysis surrogate model.

Contract: kernel(**inputs) takes FULL unsharded inputs (x [16384,5], MLP
weights, kinetic params i0/alpha) and returns the FULL [16384,2] output.
Batch is sharded 2048-per-core across 8 NeuronCores (pure data parallel);
per-core work is two 1024-sample half-streams.

Design (vs the original baseline, ~2x faster in the cost-model timeline):
- Host packs x transposed together with W1 (plus a ones row and unit
  column), so the MLP needs no on-device transposes and all layer biases
  are folded into K=6/65 matmuls via the ones-row trick.
- All matmuls run as float32r: 1 PE cycle/row for 512-wide outputs (4x
  faster than fp32) at ~fp32 precision.
- One ACT table set for the whole program (natural_log_exp_and_others,
  preloaded explicitly): sigmoid becomes exp+reciprocal, eps^1.5 becomes
  exp(1.5*ln(u+1)) with the +1 folded into the Ln bias.
- Every per-species constant (sc_k, probe factors, i0_k*e^{-t0_k},
  1/(ne F C D)) is host-computed into one small const tensor and applied
  with stride-0 broadcast APs; probe factors are pre-multiplied into
  A_k on the idle GpSimd engine (AF tiles).
- The solve runs as two independent t-half chains (derived params ->
  5-step 4-ary climb -> exact 2-point refine) that fill each other's
  ~95ns dependency bubbles on the vector engine; climb probe values use
  bf16 (the compare tolerates it; the fp32 refine restores exactness).

Algorithm: i_tot(g) is strictly decreasing in grid index g, so
argmin_g |i_tot(g) - I_TARGET| is found with a branchless 5-step 4-ary
climb (b = last g with i_tot >= target over a virtual 1024 grid; probes at
b + s*{1,2,3}) followed by an exact 2-point refinement picking the nearer
of {b, b+1} in current space.
"""

import sys

for _p in ("/opt/trn_rl_repo", "/opt/pypackages"):
    if _p not in sys.path:
        sys.path.insert(0, _p)

import numpy as np

import concourse.bacc as bacc
import concourse.bass as bass
import concourse.tile as tile
from concourse import mybir
from concourse._compat import with_exitstack

F32 = mybir.dt.float32
F32R = mybir.dt.float32r
BF16 = mybir.dt.bfloat16
I32 = mybir.dt.int32
AF = mybir.ActivationFunctionType
OP = mybir.AluOpType

# ---- problem constants (match reference.py) ----
N = 16384
NCORES = 8
NPC = N // NCORES            # 2048 samples per core
NT = NPC // 128              # 16 tiles of 128 samples
HID = 64
GRID = 1000
VMIN, VMAX = -1.25, 0.0
I_TARGET = 200.0
F_CONST = 96485.33
RT = 8.314 * 298.15
D_CO2 = 1.91e-9
C_CO2 = 34.0
E_EQ = (-0.11, 0.08, 0.0)
N_ELEC_CO2 = (2.0, 12.0)
DV = (VMAX - VMIN) / (GRID - 1)
FRT = F_CONST / RT
STEPS = [256, 64, 16, 4, 1]
_DBG_STAGE = 0

# ---- Bc (consts) tensor column map ----
COL_Z2 = 0             # [128, 16] |zlt| in lat layout (s = tau*128 + p)
COL_SC = 16            # sc_k (3)
COL_F = 19             # probe factors [j=5][w=3][k=3] (45)
COL_I0E = 64           # i0_k * exp(-t0_k) (3)
COL_CP = 67            # 1/(ne_k*F*C*D) for k=0,1; 0 for k=2 (3)
COL_LN = 70            # ln(4e-8) bias column (1)
CB = 71

NLE_SET_ID = 6         # natural_log_exp_and_others in act_func_sets


def _bc(ap, axis, n):
    """Insert a stride-0 broadcast axis of size n at `axis`."""
    shape = list(ap.shape)
    shape.insert(axis, n)
    return ap.unsqueeze(axis).broadcast_to(shape)


@with_exitstack
def _body(ctx, tc, io, first):
    nc = tc.nc
    A_d, W_d, Bc_d, out_d = io

    singles = ctx.enter_context(tc.tile_pool(name="singles", bufs=1))
    work = ctx.enter_context(tc.tile_pool(name="work", bufs=3))
    clp = ctx.enter_context(tc.tile_pool(name="clp", bufs=8))
    psum = ctx.enter_context(tc.tile_pool(name="psum", bufs=8, space="PSUM"))

    if first:
        # preload the one table set covering exp/ln/relu/copy/abs; the
        # compile-time fixpoint pass then inserts no further loads.
        nc.scalar.add_instruction(mybir.InstLoadActFuncSet(
            name=nc.get_next_instruction_name(), ins=[], outs=[],
            act_func_set_id=NLE_SET_ID))

    # ---------- input DMAs ----------
    # Biases are folded into the matmuls (K=6/65 with a ones row), so the MLP
    # needs only A and W; consts ride the parallel SWDGE queue.
    Asb = singles.tile([6, 2048 + HID + 1], F32R)  # [xT;1 | W1;b1 | e]
    nc.sync.dma_start(Asb, A_d)
    Wsb = singles.tile([65, 136], F32R)            # [W2;b2;e | W3;b3;e | W4;b4]
    nc.sync.dma_start(Wsb, W_d)
    Bsb = singles.tile([128, CB], F32)             # consts/z2
    nc.gpsimd.dma_start(Bsb, Bc_d)

    xT = Asb[:, 0:2048]
    W1 = Asb[:, 2048:2048 + HID + 1]
    W2 = Wsb[:, 0:65]
    W3 = Wsb[:, 65:130]
    W4 = Wsb[:, 130:136]
    z2 = Bsb[:, COL_Z2:COL_Z2 + NT]
    scp = Bsb[:, COL_SC:COL_SC + 3]
    fpl = Bsb[:, COL_F:COL_F + 45].rearrange("p (j w k) -> p j w k", j=5, w=3)
    i0e = Bsb[:, COL_I0E:COL_I0E + 3]
    cpl = Bsb[:, COL_CP:COL_CP + 3]
    ln4 = Bsb[:, COL_LN:COL_LN + 1]

    # climb state init (off critical path): b lives in slot 3 of the
    # per-step predicate scratch, so one reduce yields b + s*count.
    bst0 = [singles.tile([128, 4, NT // 2], F32, name=f"bs{hh}")
            for hh in range(2)]
    for hh in range(2):
        nc.vector.memset(bst0[hh][:, 3, :], -1.0)

    if _DBG_STAGE == 10:   # debug: x load only
        o = singles.tile([128, NT, 2], F32)
        nc.vector.tensor_copy(
            o, xT.rearrange("k (p t) -> p t k", t=NT)[:, :, 0:2])
        nc.sync.dma_start(out_d.rearrange("(p t) c -> p t c", t=NT), o)
        return

    # ---------- MLP (activations live as [65 hid+1, 2048 samples]) ----------
    # h row 64 = 1.0 feeds the folded bias row of the next layer's weights.
    def relu_from(ps_tiles, name):
        h = work.tile([65, 4, 512], F32R, tag=name, name=name)
        for i in range(4):
            if i % 2:
                nc.scalar.activation(
                    h[:, i, :], ps_tiles[i][0:65, :], AF.Relu, scale=1.0)
            else:
                nc.vector.tensor_scalar(
                    h[:, i, :], ps_tiles[i][0:65, :], 0.0, None, OP.max)
        return h.rearrange("p a c -> p (a c)")   # [65, 2048]

    h1ps = [psum.tile([128, 512], F32, tag="ps", name=f"h1ps{i}")
            for i in range(4)]
    for i in range(4):
        nc.tensor.matmul(h1ps[i][0:65, :], W1,
                         xT[:, i * 512:(i + 1) * 512])
    h1 = relu_from(h1ps, "h1")
    h2ps = [psum.tile([128, 512], F32, tag="ps", name=f"h2ps{i}")
            for i in range(4)]
    for i in range(4):
        nc.tensor.matmul(h2ps[i][0:65, :], W2,
                         h1[:, i * 512:(i + 1) * 512])
    h2 = relu_from(h2ps, "h2")
    h3ps = [psum.tile([128, 512], F32, tag="ps", name=f"h3ps{i}")
            for i in range(4)]
    for i in range(4):
        nc.tensor.matmul(h3ps[i][0:65, :], W3,
                         h2[:, i * 512:(i + 1) * 512])
    h3 = relu_from(h3ps, "h3")

    # layer 4: back to samples-on-partitions: lat[p, tau, j], s = tau*128+p
    # separate psum tiles per t-half so each half's lat copy waits only on
    # its own 8 matmuls (psum dep tracking is tile-granular).
    latps_h = [psum.tile([128, 48], F32, tag="ps", name=f"latps{hh}")
               for hh in range(2)]
    for t in range(NT):
        nc.tensor.matmul(latps_h[t // 8][:, (t % 8) * 6:(t % 8 + 1) * 6],
                         h3[:, t * 128:(t + 1) * 128],
                         W4)
    if _DBG_STAGE == 1:   # debug: dump lat cols 0:2 (lat layout s=t*128+p)
        lat = singles.tile([128, NT, 6], F32)
        for hh in range(2):
            nc.vector.tensor_copy(
                lat[:, hh * 8:(hh + 1) * 8, :],
                latps_h[hh].rearrange("p (t j) -> p t j", j=6))
        nc.sync.dma_start(
            out_d.rearrange("(t p) c -> p t c", p=128), lat[:, :, 0:2])
        return

    # ---------- solve: two half-streams over t ----------
    # Each half runs derived params -> 5-step 4-ary climb -> 2-point refine
    # -> store as one dependency chain. Half 0 starts as soon as the first 8
    # layer-4 matmuls land; half 1 trails by ~1us and back-fills half 0's
    # dependency bubbles on DVE/ACT (and vice versa at its own tail).
    lowprio = ctx.enter_context(tc.high_priority(offset=-1000000))
    HH = NT // 2

    def solve_half(h, lo, hi):
        W = hi - lo
        lat = singles.tile([128, W, 6], F32, name=f"lat{h}")
        nc.vector.tensor_copy(
            lat, latps_h[h].rearrange("p (t j) -> p t j", j=6))

        def latj(j):
            return lat[:, :, j]

        # u-branch: u = exp(-l1); w^1.5 = exp(1.5*ln(u+1)) (bias folds +1);
        # its tail (q/base/bp/C) lives on Pool, off the saturated DVE.
        a01 = singles.tile([128, 2, W], F32, name=f"a01{h}")
        nc.vector.tensor_tensor(a01[:, 0, :], latj(0), latj(2), OP.subtract)
        nc.vector.tensor_scalar_mul(a01[:, 1, :], latj(1), -1.0)
        eu = singles.tile([128, 2, W], F32, name=f"eu{h}")
        nc.scalar.activation(eu, a01, AF.Exp, scale=1.0)
        e1 = eu[:, 0, :]
        u = eu[:, 1, :]
        lnw = singles.tile([128, W], F32, name=f"lnw{h}")
        nc.scalar.activation(lnw, u, AF.Ln, scale=1.0, bias=1.0)
        p15 = singles.tile([128, W], F32, name=f"p15{h}")     # (1+u)^1.5
        nc.scalar.activation(p15, lnw, AF.Exp, scale=1.5)
        uinv = singles.tile([128, W], F32, name=f"uinv{h}")
        nc.vector.reciprocal(uinv, u)
        z2h = z2[:, lo:hi]
        qa = singles.tile([128, W], F32, name=f"qa{h}")
        nc.gpsimd.tensor_tensor(qa, z2h, uinv, OP.mult)
        q = singles.tile([128, W], F32, name=f"q{h}")
        nc.gpsimd.tensor_tensor(q, qa, z2h, OP.add)    # |z|*(1+1/u)
        base = singles.tile([128, W], F32, name=f"base{h}")
        nc.vector.scalar_tensor_tensor(base, e1, 4e-8, q, OP.mult, OP.add)
        bp = singles.tile([128, W], F32, name=f"bp{h}")
        nc.vector.tensor_tensor(bp, base, p15, OP.mult)
        C_all = singles.tile([128, 3, W], BF16, name=f"C{h}")  # 1/i_lim
        nc.vector.tensor_tensor(C_all, _bc(bp, 1, 3), _bc(cpl, 2, W),
                                OP.mult)

        # theta-branch: A = st / (T3*i0e)
        mm = singles.tile([128, W], F32, name=f"mm{h}")
        nc.vector.reduce_max(mm, lat[:, :, 3:6], axis=mybir.AxisListType.X)
        d3 = singles.tile([128, 3, W], F32, name=f"d3{h}")
        nc.vector.tensor_tensor(
            d3, lat[:, :, 3:6].rearrange("p t j -> p j t"), _bc(mm, 1, 3),
            OP.subtract)
        T3 = singles.tile([128, 3, W], F32, name=f"T3{h}")
        nc.scalar.activation(T3, d3, AF.Exp, scale=2.0)
        st = singles.tile([128, W], F32, name=f"st{h}")
        nc.vector.reduce_sum(st, T3.rearrange("p k t -> p t k"),
                             axis=mybir.AxisListType.X)
        Ti = singles.tile([128, 3, W], F32, name=f"Ti{h}")
        nc.vector.tensor_tensor(Ti, T3, _bc(i0e, 2, W), OP.mult)
        Ari = singles.tile([128, 3, W], F32, name=f"Ari{h}")
        nc.vector.reciprocal(Ari, Ti)
        A_all = singles.tile([128, 3, W], F32, name=f"A{h}")
        nc.vector.tensor_tensor(A_all, Ari, _bc(st, 1, 3), OP.mult)

        # probe factors folded into A per step on the idle GpSimd engine
        AFs = [None]
        for j in range(1, len(STEPS)):
            AFt = singles.tile([128, 3, 3, W], BF16, name=f"AF{j}h{h}")
            nc.gpsimd.tensor_tensor(AFt, _bc(fpl[:, j], 3, W),
                                    _bc(A_all[:], 1, 3), OP.mult)
            AFs.append(AFt)

        bsc = bst0[h]
        bh = bsc[:, 3, :]
        for j, s in enumerate(STEPS):
            # DVE is the saturated engine during the solve: route the ops
            # that tolerate GpSimd's ~1.7x cost (arg/P2/pred/badd) to Pool.
            arg = clp.tile([128, 3, W], F32, tag=f"arg{h}")
            aeng = nc.gpsimd if j == 0 else nc.vector
            aeng.tensor_tensor(arg, _bc(bh, 1, 3), _bc(scp, 2, W),
                               OP.mult)
            E0 = clp.tile([128, 3, W], BF16, tag=f"E0{h}")
            nc.scalar.activation(E0, arg, AF.Exp, scale=1.0)
            P = clp.tile([128, 3, 3, W], BF16, tag=f"P{h}")
            if j == 0:
                # step-1 probes depend only on consts: arg/E0/E front-run
                # into the MLP shadow on Pool/ACT, keeping the DVE queue
                # clear for the MLP relus.
                E = clp.tile([128, 3, 3, W], BF16, tag=f"E{h}")
                nc.gpsimd.tensor_tensor(E, _bc(E0[:], 1, 3),
                                        _bc(fpl[:, j], 3, W), OP.mult)
                nc.vector.tensor_tensor(P, E, _bc(A_all[:], 1, 3), OP.mult)
            else:
                nc.vector.tensor_tensor(P, _bc(E0[:], 1, 3), AFs[j][:],
                                        OP.mult)
            nc.vector.tensor_tensor(P, P, _bc(C_all[:], 1, 3), OP.add)
            S = clp.tile([128, 3, 3, W], BF16, tag=f"S{h}")
            nc.vector.reciprocal(S, P)
            itot = clp.tile([128, 3, W], F32, tag=f"it{h}")
            nc.vector.reduce_sum(itot, S.rearrange("p w k t -> p w t k"),
                                 axis=mybir.AxisListType.X)
            nc.vector.tensor_scalar(bsc[:, 0:3, :], itot, I_TARGET,
                                    float(s), OP.is_ge, OP.mult)
            nxt = clp.tile([128, 4, W], F32, tag=f"bs{h}")
            nc.vector.reduce_sum(nxt[:, 3, :],
                                 bsc.rearrange("p w t -> p t w"),
                                 axis=mybir.AxisListType.X)
            bsc = nxt
            bh = bsc[:, 3, :]

        if _DBG_STAGE == 3:
            nc.sync.dma_start(
                out_d.rearrange("p (t c) -> p t c", c=2)[:, lo:hi, 0], bh)
            return

        # refine: evaluate the two bracketing real-grid points
        g01 = clp.tile([128, 2, W], F32, tag=f"g{h}")
        nc.vector.tensor_scalar(g01[:, 0, :], bh, 0.0, float(GRID - 1),
                                OP.max, OP.min)
        nc.vector.tensor_scalar(g01[:, 1, :], bh, 1.0, float(GRID - 1),
                                OP.add, OP.min)
        argP = clp.tile([128, 2, 3, W], F32, tag=f"aP{h}")
        nc.vector.tensor_tensor(argP, _bc(g01[:], 2, 3),
                                _bc(_bc(scp, 2, W), 1, 2), OP.mult)
        EP = clp.tile([128, 2, 3, W], F32, tag=f"EP{h}")
        nc.scalar.activation(EP, argP, AF.Exp, scale=1.0)
        PP = clp.tile([128, 2, 3, W], F32, tag=f"PP{h}")
        nc.vector.tensor_tensor(PP, EP, _bc(A_all[:], 1, 2), OP.mult)
        nc.vector.tensor_tensor(PP, PP, _bc(C_all[:], 1, 2), OP.add)
        SP = clp.tile([128, 2, 3, W], F32, tag=f"SP{h}")
        nc.vector.reciprocal(SP, PP)
        itP = clp.tile([128, 2, W], F32, tag=f"iP{h}")
        nc.vector.reduce_sum(itP, SP.rearrange("p g k t -> p g t k"),
                             axis=mybir.AxisListType.X)
        d0 = clp.tile([128, W], F32, tag=f"d0{h}")
        # d0 <= d1  <=>  it0+it1 <= 2*target (it0 >= it1, monotone)
        nc.vector.tensor_tensor(d0, itP[:, 0, :], itP[:, 1, :], OP.add)
        pick0 = clp.tile([128, W], I32, tag=f"pk{h}")
        nc.vector.tensor_scalar(pick0, d0, 2.0 * I_TARGET, None, OP.is_le)
        # FE candidates for both bracket points, then one select per output
        feR = clp.tile([128, 2, W], F32, tag=f"fR{h}")
        nc.vector.reciprocal(feR, itP)
        cand = clp.tile([128, 2, 2, W], F32, tag=f"cd{h}")
        nc.vector.tensor_tensor(cand, SP[:, :, 0:2, :], _bc(feR[:], 2, 2),
                                OP.mult)
        fe = singles.tile([128, W, 2], F32, name=f"fe{h}")
        nc.vector.select(fe[:, :, 0], pick0, cand[:, 0, 1, :],
                         cand[:, 1, 1, :])
        nc.vector.select(fe[:, :, 1], pick0, cand[:, 0, 0, :],
                         cand[:, 1, 0, :])
        # per-half store [p, t, c]; host un-permutes (s = t*128 + p).
        # h0's descgen overlaps h1's refine (the halves finish ~1us apart).
        nc.sync.dma_start(
            out_d.rearrange("p (t c) -> p t c", c=2)[:, lo:hi], fe)

    with nc.allow_low_precision(
            reason="bf16 climb probe values; exact fp32 refine follows"):
        solve_half(0, 0, HH)
        # half 1 ends the program: let it win engine contention against
        # half 0's tail so the two halves finish balanced.
        with tc.high_priority(offset=400):
            solve_half(1, HH, NT)


def _build(i0, alpha, reps=1):
    nc = bacc.Bacc("TRN2", target_bir_lowering=False, debug=False)
    A_d = nc.dram_tensor("xw", [6, 2048 + HID + 1], F32R,
                     kind="ExternalInput").ap()
    W_d = nc.dram_tensor("ww", [65, 136], F32R, kind="ExternalInput").ap()
    Bc_d = nc.dram_tensor("wc", [128, CB], F32, kind="ExternalInput").ap()
    out_d = nc.dram_tensor("out", [128, NT * 2], F32, kind="ExternalOutput").ap()
    io = (A_d, W_d, Bc_d, out_d)
    with tile.TileContext(nc) as tc:
        for r in range(reps):
            _body(tc, io, r == 0)
    nc.compile()
    return nc


def _host_pack(x, W1, b1, W2, b2, W3, b3, W4, b4, i0, alpha):
    """Per-core A ([5, 2112] = xT|W1) and B ([128, CB]) host tensors."""
    x = np.ascontiguousarray(np.asarray(x, np.float32))
    i0 = np.asarray(i0, np.float64)
    alpha = np.asarray(alpha, np.float64)
    sc = alpha * FRT * DV
    t0 = alpha * FRT * (VMIN - np.asarray(E_EQ, np.float64))

    Wc = np.zeros((65, 136), np.float32)
    Wc[0:64, 0:64] = np.asarray(W2, np.float32)
    Wc[64, 0:64] = np.asarray(b2, np.float32)
    Wc[64, 64] = 1.0
    Wc[0:64, 65:129] = np.asarray(W3, np.float32)
    Wc[64, 65:129] = np.asarray(b3, np.float32)
    Wc[64, 129] = 1.0
    Wc[0:64, 130:136] = np.asarray(W4, np.float32)
    Wc[64, 130:136] = np.asarray(b4, np.float32)
    Wc = np.ascontiguousarray(Wc)

    Bc = np.zeros((128, CB), np.float32)
    Bc[:, COL_SC:COL_SC + 3] = sc.astype(np.float32)[None, :]
    f = np.empty((5, 3, 3), np.float64)
    for j, s in enumerate(STEPS):
        for wp in range(3):
            f[j, wp, :] = np.exp(sc * s * (wp + 1))
    Bc[:, COL_F:COL_F + 45] = f.reshape(-1).astype(np.float32)[None, :]
    Bc[:, COL_I0E:COL_I0E + 3] = (i0 * np.exp(-t0)).astype(np.float32)[None, :]
    cp = np.zeros(3, np.float64)
    cp[0] = 1.0 / (np.float32(N_ELEC_CO2[0]) * F_CONST * C_CO2 * D_CO2)
    cp[1] = 1.0 / (np.float32(N_ELEC_CO2[1]) * F_CONST * C_CO2 * D_CO2)
    Bc[:, COL_CP:COL_CP + 3] = cp.astype(np.float32)[None, :]
    Bc[:, COL_LN] = np.float32(np.log(4e-8))

    As, Bs = [], []
    for c in range(NCORES):
        xc = x[c * NPC:(c + 1) * NPC]              # [2048, 5]
        A = np.zeros((6, 2048 + HID + 1), np.float32)
        A[0:5, 0:2048] = xc.T
        A[5, 0:2048] = 1.0
        A[0:5, 2048:2048 + HID] = np.asarray(W1, np.float32)
        A[5, 2048:2048 + HID] = np.asarray(b1, np.float32)
        A[5, 2048 + HID] = 1.0
        B = Bc.copy()
        # z2[p, tau] = |x[tau*128 + p, 3]|  (lat layout)
        B[:, COL_Z2:COL_Z2 + NT] = np.abs(xc[:, 3]).reshape(NT, 128).T
        As.append(np.ascontiguousarray(A))
        Bs.append(np.ascontiguousarray(B))
    return As, Wc, Bs


_CACHE = {}


def kernel(x, W1, b1, W2, b2, W3, b3, W4, b4, i0, alpha):
    from concourse.bass_utils import run_bass_kernel_spmd

    if "nc" not in _CACHE:
        _CACHE["nc"] = _build(i0, alpha)
    nc = _CACHE["nc"]
    As, Wc, Bs = _host_pack(x, W1, b1, W2, b2, W3, b3, W4, b4, i0, alpha)
    in_maps = [{"xw": As[c], "ww": Wc, "wc": Bs[c]}
               for c in range(NCORES)]
    res = run_bass_kernel_spmd(nc, in_maps, core_ids=list(range(NCORES)))
    outs = []
    for c in range(NCORES):
        o = res.results[c]["out"]                  # [128, NT*2]: fe[p, t, c]
        outs.append(o.reshape(128, NT, 2).transpose(1, 0, 2).reshape(NPC, 2))
    return np.concatenate(outs, axis=0)


# revision 9
# speedup vs baseline: 6.2244x; 6.2244x over previous
"""Trainium2 Bass kernel for the CO2-electrolysis surrogate model.

Contract: kernel(**inputs) takes FULL unsharded inputs (x [16384,5], MLP
weights, kinetic params i0/alpha) and returns the FULL [16384,2] output.
Batch is sharded 2048-per-core across 8 NeuronCores (pure data parallel);
per-core work is two 1024-sample half-streams.

Design (vs the original baseline, ~2x faster in the cost-model timeline):
- Host packs x transposed together with W1 (plus a ones row and unit
  column), so the MLP needs no on-device transposes and all layer biases
  are folded into K=6/65 matmuls via the ones-row trick.
- All matmuls run as float32r: 1 PE cycle/row for 512-wide outputs (4x
  faster than fp32) at ~fp32 precision.
- One ACT table set for the whole program (natural_log_exp_and_others,
  preloaded explicitly): sigmoid becomes exp+reciprocal, eps^1.5 becomes
  exp(1.5*ln(u+1)) with the +1 folded into the Ln bias.
- Every per-species constant (sc_k, probe factors, i0_k*e^{-t0_k},
  1/(ne F C D)) is host-computed into one small const tensor and applied
  with stride-0 broadcast APs; probe factors are pre-multiplied into
  A_k on the idle GpSimd engine (AF tiles).
- The solve runs as two independent t-half chains (derived params ->
  3-step 16/8/8-ary climb -> exact 2-point refine) that fill each other's
  ~95ns dependency bubbles on the vector engine; climb probe values use
  bf16 (the compare tolerates it; the fp32 refine restores exactness).
  The climb state b rides in the last slot of each step's predicate
  scratch so a single reduce yields b + stride*count.

Algorithm: i_tot(g) is strictly decreasing in grid index g, so
argmin_g |i_tot(g) - I_TARGET| is found with a branchless 3-step climb
(strides 64/8/1 with 15/7/7 probes; b = last g with i_tot >= target over
a virtual 1024 grid) followed by an exact 2-point refinement picking the
nearer of {b, b+1} in current space.
"""

import sys

for _p in ("/opt/trn_rl_repo", "/opt/pypackages"):
    if _p not in sys.path:
        sys.path.insert(0, _p)

import numpy as np

import concourse.bacc as bacc
import concourse.bass as bass
import concourse.tile as tile
from concourse import mybir
from concourse._compat import with_exitstack

F32 = mybir.dt.float32
F32R = mybir.dt.float32r
BF16 = mybir.dt.bfloat16
I32 = mybir.dt.int32
AF = mybir.ActivationFunctionType
OP = mybir.AluOpType

# ---- problem constants (match reference.py) ----
N = 16384
NCORES = 8
NPC = N // NCORES            # 2048 samples per core
NT = NPC // 128              # 16 tiles of 128 samples
HID = 64
GRID = 1000
VMIN, VMAX = -1.25, 0.0
I_TARGET = 200.0
F_CONST = 96485.33
RT = 8.314 * 298.15
D_CO2 = 1.91e-9
C_CO2 = 34.0
E_EQ = (-0.11, 0.08, 0.0)
N_ELEC_CO2 = (2.0, 12.0)
DV = (VMAX - VMIN) / (GRID - 1)
FRT = F_CONST / RT
STEPS = [(64, 15), (8, 7), (1, 7)]   # (stride, probes)
_DBG_STAGE = 0

# ---- Bc (consts) tensor column map ----
COL_Z2 = 0             # [128, 16] |zlt| in lat layout (s = tau*128 + p)
COL_SC = 16            # sc_k (3)
COL_F = 19             # probe factors, ragged [j][w_j][k] (87)
NWS = [w for (_s, w) in STEPS]
COL_FJ = [COL_F + 3 * sum(NWS[:j]) for j in range(len(STEPS))]
COL_I0E = COL_F + 3 * sum(NWS)      # i0_k * exp(-t0_k) (3)
COL_CP = COL_I0E + 3   # 1/(ne_k*F*C*D) for k=0,1; 0 for k=2 (3)
COL_LN = COL_CP + 3    # ln(4e-8) bias column (1)
CB = COL_LN + 1

NLE_SET_ID = 6         # natural_log_exp_and_others in act_func_sets


def _bc(ap, axis, n):
    """Insert a stride-0 broadcast axis of size n at `axis`."""
    shape = list(ap.shape)
    shape.insert(axis, n)
    return ap.unsqueeze(axis).broadcast_to(shape)


@with_exitstack
def _body(ctx, tc, io, first):
    nc = tc.nc
    A_d, W_d, Bc_d, out_d = io

    singles = ctx.enter_context(tc.tile_pool(name="singles", bufs=1))
    work = ctx.enter_context(tc.tile_pool(name="work", bufs=3))
    clp = ctx.enter_context(tc.tile_pool(name="clp", bufs=8))
    psum = ctx.enter_context(tc.tile_pool(name="psum", bufs=8, space="PSUM"))

    if first:
        # preload the one table set covering exp/ln/relu/copy/abs; the
        # compile-time fixpoint pass then inserts no further loads.
        nc.scalar.add_instruction(mybir.InstLoadActFuncSet(
            name=nc.get_next_instruction_name(), ins=[], outs=[],
            act_func_set_id=NLE_SET_ID))

    # ---------- input DMAs ----------
    # Biases are folded into the matmuls (K=6/65 with a ones row), so the MLP
    # needs only A and W; consts ride the parallel SWDGE queue.
    Asb = singles.tile([6, 2048 + HID + 1], F32R)  # [xT;1 | W1;b1 | e]
    nc.sync.dma_start(Asb, A_d)
    Wsb = singles.tile([65, 136], F32R)            # [W2;b2;e | W3;b3;e | W4;b4]
    nc.sync.dma_start(Wsb, W_d)
    Bsb = singles.tile([128, CB], F32)             # consts/z2
    nc.gpsimd.dma_start(Bsb, Bc_d)

    xT = Asb[:, 0:2048]
    W1 = Asb[:, 2048:2048 + HID + 1]
    W2 = Wsb[:, 0:65]
    W3 = Wsb[:, 65:130]
    W4 = Wsb[:, 130:136]
    z2 = Bsb[:, COL_Z2:COL_Z2 + NT]
    scp = Bsb[:, COL_SC:COL_SC + 3]
    fpl = [Bsb[:, COL_FJ[j]:COL_FJ[j] + 3 * NWS[j]].rearrange(
        "p (w k) -> p w k", k=3) for j in range(len(STEPS))]
    i0e = Bsb[:, COL_I0E:COL_I0E + 3]
    cpl = Bsb[:, COL_CP:COL_CP + 3]
    ln4 = Bsb[:, COL_LN:COL_LN + 1]

    # climb state init (off critical path): b lives in the last slot of the
    # per-step predicate scratch, so one reduce yields b + s*count.
    NW0 = NWS[0]
    bst0 = [singles.tile([128, NW0 + 1, NT // 2], F32, name=f"bs{hh}")
            for hh in range(2)]
    for hh in range(2):
        nc.vector.memset(bst0[hh][:, NW0, :], -1.0)

    if _DBG_STAGE == 10:   # debug: x load only
        o = singles.tile([128, NT, 2], F32)
        nc.vector.tensor_copy(
            o, xT.rearrange("k (p t) -> p t k", t=NT)[:, :, 0:2])
        nc.sync.dma_start(out_d.rearrange("(p t) c -> p t c", t=NT), o)
        return

    # ---------- MLP (activations live as [65 hid+1, 2048 samples]) ----------
    # h row 64 = 1.0 feeds the folded bias row of the next layer's weights.
    def relu_from(ps_tiles, name):
        h = work.tile([65, 4, 512], F32R, tag=name, name=name)
        for i in range(4):
            if i % 2:
                nc.scalar.activation(
                    h[:, i, :], ps_tiles[i][0:65, :], AF.Relu, scale=1.0)
            else:
                nc.vector.tensor_scalar(
                    h[:, i, :], ps_tiles[i][0:65, :], 0.0, None, OP.max)
        return h.rearrange("p a c -> p (a c)")   # [65, 2048]

    h1ps = [psum.tile([128, 512], F32, tag="ps", name=f"h1ps{i}")
            for i in range(4)]
    for i in range(4):
        nc.tensor.matmul(h1ps[i][0:65, :], W1,
                         xT[:, i * 512:(i + 1) * 512])
    h1 = relu_from(h1ps, "h1")
    h2ps = [psum.tile([128, 512], F32, tag="ps", name=f"h2ps{i}")
            for i in range(4)]
    for i in range(4):
        nc.tensor.matmul(h2ps[i][0:65, :], W2,
                         h1[:, i * 512:(i + 1) * 512])
    h2 = relu_from(h2ps, "h2")
    h3ps = [psum.tile([128, 512], F32, tag="ps", name=f"h3ps{i}")
            for i in range(4)]
    for i in range(4):
        nc.tensor.matmul(h3ps[i][0:65, :], W3,
                         h2[:, i * 512:(i + 1) * 512])
    h3 = relu_from(h3ps, "h3")

    # layer 4: back to samples-on-partitions: lat[p, tau, j], s = tau*128+p
    # separate psum tiles per t-half so each half's lat copy waits only on
    # its own 8 matmuls (psum dep tracking is tile-granular).
    latps_h = [psum.tile([128, 48], F32, tag="ps", name=f"latps{hh}")
               for hh in range(2)]
    for t in range(NT):
        nc.tensor.matmul(latps_h[t // 8][:, (t % 8) * 6:(t % 8 + 1) * 6],
                         h3[:, t * 128:(t + 1) * 128],
                         W4)
    if _DBG_STAGE == 1:   # debug: dump lat cols 0:2 (lat layout s=t*128+p)
        lat = singles.tile([128, NT, 6], F32)
        for hh in range(2):
            nc.vector.tensor_copy(
                lat[:, hh * 8:(hh + 1) * 8, :],
                latps_h[hh].rearrange("p (t j) -> p t j", j=6))
        nc.sync.dma_start(
            out_d.rearrange("(t p) c -> p t c", p=128), lat[:, :, 0:2])
        return

    # ---------- solve: two half-streams over t ----------
    # Each half runs derived params -> 5-step 4-ary climb -> 2-point refine
    # -> store as one dependency chain. Half 0 starts as soon as the first 8
    # layer-4 matmuls land; half 1 trails by ~1us and back-fills half 0's
    # dependency bubbles on DVE/ACT (and vice versa at its own tail).
    lowprio = ctx.enter_context(tc.high_priority(offset=-1000000))
    HH = NT // 2

    def solve_half(h, lo, hi):
        W = hi - lo
        lat = singles.tile([128, W, 6], F32, name=f"lat{h}")
        nc.vector.tensor_copy(
            lat, latps_h[h].rearrange("p (t j) -> p t j", j=6))

        def latj(j):
            return lat[:, :, j]

        # u-branch: u = exp(-l1); w^1.5 = exp(1.5*ln(u+1)) (bias folds +1);
        # its tail (q/base/bp/C) lives on Pool, off the saturated DVE.
        a01 = singles.tile([128, 2, W], F32, name=f"a01{h}")
        nc.vector.tensor_tensor(a01[:, 0, :], latj(0), latj(2), OP.subtract)
        nc.vector.tensor_scalar_mul(a01[:, 1, :], latj(1), -1.0)
        eu = singles.tile([128, 2, W], F32, name=f"eu{h}")
        nc.scalar.activation(eu, a01, AF.Exp, scale=1.0)
        e1 = eu[:, 0, :]
        u = eu[:, 1, :]
        lnw = singles.tile([128, W], F32, name=f"lnw{h}")
        nc.scalar.activation(lnw, u, AF.Ln, scale=1.0, bias=1.0)
        p15 = singles.tile([128, W], F32, name=f"p15{h}")     # (1+u)^1.5
        nc.scalar.activation(p15, lnw, AF.Exp, scale=1.5)
        uinv = singles.tile([128, W], F32, name=f"uinv{h}")
        nc.vector.reciprocal(uinv, u)
        z2h = z2[:, lo:hi]
        qa = singles.tile([128, W], F32, name=f"qa{h}")
        nc.gpsimd.tensor_tensor(qa, z2h, uinv, OP.mult)
        q = singles.tile([128, W], F32, name=f"q{h}")
        nc.gpsimd.tensor_tensor(q, qa, z2h, OP.add)    # |z|*(1+1/u)
        base = singles.tile([128, W], F32, name=f"base{h}")
        nc.vector.scalar_tensor_tensor(base, e1, 4e-8, q, OP.mult, OP.add)
        bp = singles.tile([128, W], F32, name=f"bp{h}")
        nc.vector.tensor_tensor(bp, base, p15, OP.mult)
        C_all = singles.tile([128, 3, W], BF16, name=f"C{h}")  # 1/i_lim
        nc.vector.tensor_tensor(C_all, _bc(bp, 1, 3), _bc(cpl, 2, W),
                                OP.mult)

        # theta-branch: A = st / (T3*i0e)
        mm = singles.tile([128, W], F32, name=f"mm{h}")
        nc.vector.reduce_max(mm, lat[:, :, 3:6], axis=mybir.AxisListType.X)
        d3 = singles.tile([128, 3, W], F32, name=f"d3{h}")
        nc.vector.tensor_tensor(
            d3, lat[:, :, 3:6].rearrange("p t j -> p j t"), _bc(mm, 1, 3),
            OP.subtract)
        T3 = singles.tile([128, 3, W], F32, name=f"T3{h}")
        nc.scalar.activation(T3, d3, AF.Exp, scale=2.0)
        st = singles.tile([128, W], F32, name=f"st{h}")
        nc.vector.reduce_sum(st, T3.rearrange("p k t -> p t k"),
                             axis=mybir.AxisListType.X)
        Ti = singles.tile([128, 3, W], F32, name=f"Ti{h}")
        nc.vector.tensor_tensor(Ti, T3, _bc(i0e, 2, W), OP.mult)
        Ari = singles.tile([128, 3, W], F32, name=f"Ari{h}")
        nc.vector.reciprocal(Ari, Ti)
        A_all = singles.tile([128, 3, W], F32, name=f"A{h}")
        nc.vector.tensor_tensor(A_all, Ari, _bc(st, 1, 3), OP.mult)
        A_bf = singles.tile([128, 3, W], BF16, name=f"Ab{h}")
        nc.vector.tensor_copy(A_bf, A_all)

        # probe factors folded into A per step on the idle GpSimd engine
        AFs = [None]
        for j in range(1, len(STEPS)):
            nw = NWS[j]
            AFt = singles.tile([128, nw, 3, W], BF16, name=f"AF{j}h{h}")
            nc.gpsimd.tensor_tensor(AFt, _bc(fpl[j], 3, W),
                                    _bc(A_all[:], 1, nw), OP.mult)
            AFs.append(AFt)

        bsc = bst0[h]
        for j, (s, nw) in enumerate(STEPS):
            bh = bsc[:, bsc.shape[1] - 1, :]
            arg = clp.tile([128, 3, W], F32, tag=f"arg{h}")
            aeng = nc.gpsimd if j == 0 else nc.vector
            aeng.tensor_tensor(arg, _bc(bh, 1, 3), _bc(scp, 2, W),
                               OP.mult)
            E0 = clp.tile([128, 3, W], BF16, tag=f"E0{h}")
            nc.scalar.activation(E0, arg, AF.Exp, scale=1.0)
            P = clp.tile([128, nw, 3, W], BF16, tag=f"P{h}{j}")
            if j == 0:
                # step-1 probes depend only on consts: arg/E0/E front-run
                # into the MLP shadow on Pool/ACT, keeping the DVE queue
                # clear for the MLP relus.
                E = clp.tile([128, nw, 3, W], BF16, tag=f"E{h}")
                nc.gpsimd.tensor_tensor(E, _bc(E0[:], 1, nw),
                                        _bc(fpl[j], 3, W), OP.mult)
                nc.vector.tensor_tensor(P, E, _bc(A_bf[:], 1, nw), OP.mult)
            else:
                nc.vector.tensor_tensor(P, _bc(E0[:], 1, nw), AFs[j][:],
                                        OP.mult)
            nc.vector.tensor_tensor(P, P, _bc(C_all[:], 1, nw), OP.add)
            S = clp.tile([128, nw, 3, W], BF16, tag=f"S{h}{j}")
            nc.vector.reciprocal(S, P)
            itot = clp.tile([128, nw, W], BF16, tag=f"it{h}{j}")
            nc.vector.tensor_tensor(itot, S[:, :, 0, :], S[:, :, 1, :],
                                    OP.add)
            nc.vector.tensor_tensor(itot, itot, S[:, :, 2, :], OP.add)
            nc.vector.tensor_scalar(bsc[:, 0:nw, :], itot, I_TARGET,
                                    float(s), OP.is_ge, OP.mult)
            if j + 1 < len(STEPS):
                nxt = clp.tile([128, NWS[j + 1] + 1, W], F32,
                               tag=f"bs{h}{j}")
                tgt = nxt[:, NWS[j + 1], :]
            else:
                nxt = clp.tile([128, 1, W], F32, tag=f"bf{h}")
                tgt = nxt[:, 0, :]
            nc.vector.reduce_sum(tgt, bsc.rearrange("p w t -> p t w"),
                                 axis=mybir.AxisListType.X)
            bsc = nxt
        bh = bsc[:, 0, :]

        if _DBG_STAGE == 3:
            nc.sync.dma_start(
                out_d.rearrange("p (t c) -> p t c", c=2)[:, lo:hi, 0], bh)
            return

        # refine: evaluate the two bracketing real-grid points
        g01 = clp.tile([128, 2, W], F32, tag=f"g{h}")
        nc.vector.tensor_scalar(g01[:, 0, :], bh, 0.0, float(GRID - 1),
                                OP.max, OP.min)
        nc.vector.tensor_scalar(g01[:, 1, :], bh, 1.0, float(GRID - 1),
                                OP.add, OP.min)
        argP = clp.tile([128, 2, 3, W], F32, tag=f"aP{h}")
        nc.vector.tensor_tensor(argP, _bc(g01[:], 2, 3),
                                _bc(_bc(scp, 2, W), 1, 2), OP.mult)
        EP = clp.tile([128, 2, 3, W], F32, tag=f"EP{h}")
        nc.scalar.activation(EP, argP, AF.Exp, scale=1.0)
        PP = clp.tile([128, 2, 3, W], F32, tag=f"PP{h}")
        nc.vector.tensor_tensor(PP, EP, _bc(A_all[:], 1, 2), OP.mult)
        nc.vector.tensor_tensor(PP, PP, _bc(C_all[:], 1, 2), OP.add)
        SP = clp.tile([128, 2, 3, W], F32, tag=f"SP{h}")
        nc.vector.reciprocal(SP, PP)
        itP = clp.tile([128, 2, W], F32, tag=f"iP{h}")
        nc.vector.reduce_sum(itP, SP.rearrange("p g k t -> p g t k"),
                             axis=mybir.AxisListType.X)
        d0 = clp.tile([128, W], F32, tag=f"d0{h}")
        # d0 <= d1  <=>  it0+it1 <= 2*target (it0 >= it1, monotone)
        nc.vector.tensor_tensor(d0, itP[:, 0, :], itP[:, 1, :], OP.add)
        pick0 = clp.tile([128, W], I32, tag=f"pk{h}")
        nc.vector.tensor_scalar(pick0, d0, 2.0 * I_TARGET, None, OP.is_le)
        # FE candidates for both bracket points, then one select per output
        feR = clp.tile([128, 2, W], F32, tag=f"fR{h}")
        nc.vector.reciprocal(feR, itP)
        cand = clp.tile([128, 2, 2, W], F32, tag=f"cd{h}")
        nc.vector.tensor_tensor(cand, SP[:, :, 0:2, :], _bc(feR[:], 2, 2),
                                OP.mult)
        fe = singles.tile([128, W, 2], F32, name=f"fe{h}")
        nc.vector.select(fe[:, :, 0], pick0, cand[:, 0, 1, :],
                         cand[:, 1, 1, :])
        nc.vector.select(fe[:, :, 1], pick0, cand[:, 0, 0, :],
                         cand[:, 1, 0, :])
        # per-half store [p, t, c]; host un-permutes (s = t*128 + p).
        # h0's descgen overlaps h1's refine (the halves finish ~1us apart).
        nc.sync.dma_start(
            out_d.rearrange("p (t c) -> p t c", c=2)[:, lo:hi], fe)

    with nc.allow_low_precision(
            reason="bf16 climb probe values; exact fp32 refine follows"):
        solve_half(0, 0, HH)
        # half 1 ends the program: let it win engine contention against
        # half 0's tail so the two halves finish balanced.
        with tc.high_priority(offset=400):
            solve_half(1, HH, NT)


def _build(i0, alpha, reps=1):
    nc = bacc.Bacc("TRN2", target_bir_lowering=False, debug=False)
    A_d = nc.dram_tensor("xw", [6, 2048 + HID + 1], F32R,
                     kind="ExternalInput").ap()
    W_d = nc.dram_tensor("ww", [65, 136], F32R, kind="ExternalInput").ap()
    Bc_d = nc.dram_tensor("wc", [128, CB], F32, kind="ExternalInput").ap()
    out_d = nc.dram_tensor("out", [128, NT * 2], F32, kind="ExternalOutput").ap()
    io = (A_d, W_d, Bc_d, out_d)
    with tile.TileContext(nc) as tc:
        for r in range(reps):
            _body(tc, io, r == 0)
    nc.compile()
    return nc


def _host_pack(x, W1, b1, W2, b2, W3, b3, W4, b4, i0, alpha):
    """Per-core A ([5, 2112] = xT|W1) and B ([128, CB]) host tensors."""
    x = np.ascontiguousarray(np.asarray(x, np.float32))
    i0 = np.asarray(i0, np.float64)
    alpha = np.asarray(alpha, np.float64)
    sc = alpha * FRT * DV
    t0 = alpha * FRT * (VMIN - np.asarray(E_EQ, np.float64))

    Wc = np.zeros((65, 136), np.float32)
    Wc[0:64, 0:64] = np.asarray(W2, np.float32)
    Wc[64, 0:64] = np.asarray(b2, np.float32)
    Wc[64, 64] = 1.0
    Wc[0:64, 65:129] = np.asarray(W3, np.float32)
    Wc[64, 65:129] = np.asarray(b3, np.float32)
    Wc[64, 129] = 1.0
    Wc[0:64, 130:136] = np.asarray(W4, np.float32)
    Wc[64, 130:136] = np.asarray(b4, np.float32)
    Wc = np.ascontiguousarray(Wc)

    Bc = np.zeros((128, CB), np.float32)
    Bc[:, COL_SC:COL_SC + 3] = sc.astype(np.float32)[None, :]
    fcols = []
    for (s, nw) in STEPS:
        for wp in range(nw):
            fcols.append(np.exp(sc * s * (wp + 1)))
    farr = np.stack(fcols).reshape(-1)            # [sum(nw)*3]
    Bc[:, COL_F:COL_F + farr.size] = farr.astype(np.float32)[None, :]
    Bc[:, COL_I0E:COL_I0E + 3] = (i0 * np.exp(-t0)).astype(np.float32)[None, :]
    cp = np.zeros(3, np.float64)
    cp[0] = 1.0 / (np.float32(N_ELEC_CO2[0]) * F_CONST * C_CO2 * D_CO2)
    cp[1] = 1.0 / (np.float32(N_ELEC_CO2[1]) * F_CONST * C_CO2 * D_CO2)
    Bc[:, COL_CP:COL_CP + 3] = cp.astype(np.float32)[None, :]
    Bc[:, COL_LN] = np.float32(np.log(4e-8))

    As, Bs = [], []
    for c in range(NCORES):
        xc = x[c * NPC:(c + 1) * NPC]              # [2048, 5]
        A = np.zeros((6, 2048 + HID + 1), np.float32)
        A[0:5, 0:2048] = xc.T
        A[5, 0:2048] = 1.0
        A[0:5, 2048:2048 + HID] = np.asarray(W1, np.float32)
        A[5, 2048:2048 + HID] = np.asarray(b1, np.float32)
        A[5, 2048 + HID] = 1.0
        B = Bc.copy()
        # z2[p, tau] = |x[tau*128 + p, 3]|  (lat layout)
        B[:, COL_Z2:COL_Z2 + NT] = np.abs(xc[:, 3]).reshape(NT, 128).T
        As.append(np.ascontiguousarray(A))
        Bs.append(np.ascontiguousarray(B))
    return As, Wc, Bs


_CACHE = {}


def kernel(x, W1, b1, W2, b2, W3, b3, W4, b4, i0, alpha):
    from concourse.bass_utils import run_bass_kernel_spmd

    if "nc" not in _CACHE:
        _CACHE["nc"] = _build(i0, alpha)
    nc = _CACHE["nc"]
    As, Wc, Bs = _host_pack(x, W1, b1, W2, b2, W3, b3, W4, b4, i0, alpha)
    in_maps = [{"xw": As[c], "ww": Wc, "wc": Bs[c]}
               for c in range(NCORES)]
    res = run_bass_kernel_spmd(nc, in_maps, core_ids=list(range(NCORES)))
    outs = []
    for c in range(NCORES):
        o = res.results[c]["out"]                  # [128, NT*2]: fe[p, t, c]
        outs.append(o.reshape(128, NT, 2).transpose(1, 0, 2).reshape(NPC, 2))
    return np.concatenate(outs, axis=0)
